# revision 17
# baseline (speedup 1.0000x reference)
"""Trainium2 Bass kernel for AdaptSelfAttention (Transformer-XL style relative
position attention).

Shapes (hardcoded): B=4, L=1024, H=512, NH=8, HD=64.
Sharding: 32 (batch, head) pairs -> 8 cores; core c handles batch c//2 and the
4-head group c%2 (hidden slice of 256 columns).

Math per (b, n):
  q = query @ Wq + bq   (per-head slice)          [L, 64]
  v = value @ Wv + bv                              [L, 64]
  k = key slice                                    [L, 64]
  rel = emb @ Wr + br                              [2L, 64]  (emb = sinusoid const)
  S[q_,k_] = (q+rrb).k  +  (q+rwb).rel[L+k_-q_]  +  k.rel[L+q_-k_]  + c2[k_]
       (c2[k_] = k.br ; the q-side br term is constant per row -> softmax-
        invariant, dropped)
  out = softmax_k(S with key-mask) @ v

v2 design (all shift scratch kept in SBUF; no DRAM round trip):
  - B_[q,l] / E_[k,l] windows ([128, 8*1152] bf16 per tensor per pair) are
    produced by PE into PSUM and copied straight into SBUF scratch.
  - Skewed (diagonal-AP) SBUF->SBUF DMAs, one per 128-row block, give
    bdall[q, k] = BD_sh and eshall = E_sh^T without touching HBM.
  - ki-loop: PE transposes BD blocks into a bf16 PSUM tile; one DVE add
    (BDT half + esh half) initializes the S^T PSUM; the AC matmul accumulates
    on top (start=False); exp on ACT with the key-mask/c2 bias; AV with
    lhsT=[v|1] so the softmax denominator is row 64 of the output PSUM.
  - finalize: DVE divide by the broadcast denominator row; output written
    [64, 1024] (transposed), host restores orientation.
Pairs are software-pipelined: pair p+1's produce+skew interleave pair p's
ki-loop at two produce-blocks per ki.
"""

import math
import sys

import numpy as np

sys.path.insert(0, "/opt/trn_rl_repo")

import concourse.bass as bass
import concourse.tile as tile
from concourse import bacc, mybir
from concourse.bass_utils import run_bass_kernel_spmd

import ml_dtypes

BF16 = ml_dtypes.bfloat16

B, L, H, NH, HD = 4, 1024, 512, 8, 64
PITCH = 1152  # stored l-window width per 128-row block of the B_/E_ scratch
SCRW = PITCH * 8
NEG = -1e30


def _get_embedding(max_len, dim):
    half = dim // 2
    freq = np.exp(np.arange(half, dtype=np.float64) * (-math.log(10000.0) / (half - 1)))
    pos = np.arange(-max_len, max_len, dtype=np.float64)
    ang = pos[:, None] * freq[None, :]
    return np.concatenate([np.sin(ang), np.cos(ang)], axis=1)


def build_body(tc, ins, outs):
    """Emit the per-core kernel. ins/outs: dicts of bass.AP over DRAM."""
    nc = tc.nc
    f32 = mybir.dt.float32
    bf16 = mybir.dt.bfloat16
    Ident = mybir.ActivationFunctionType.Identity
    Exp = mybir.ActivationFunctionType.Exp

    from contextlib import ExitStack

    ctx = ExitStack()
    with ctx:
        # ---- pools ----
        io = ctx.enter_context(tc.tile_pool(name="io", bufs=1))
        persist = ctx.enter_context(tc.tile_pool(name="persist", bufs=1))
        scrp = ctx.enter_context(tc.tile_pool(name="scrp", bufs=1))   # scratch windows
        skp = ctx.enter_context(tc.tile_pool(name="skp", bufs=2))     # skew landing
        pp = ctx.enter_context(tc.tile_pool(name="pp", bufs=4))       # exp outputs
        finp = ctx.enter_context(tc.tile_pool(name="finp", bufs=1))
        denp = ctx.enter_context(tc.tile_pool(name="denp", bufs=1))
        sml = ctx.enter_context(tc.tile_pool(name="sml", bufs=4))
        # PSUM: stg 2x[128,512]f32=2 banks; psS 3x[128,512]f32=3;
        # psB 2x[128,512]bf16=1; psO 1x[65,1024]f32=2  -> 8 banks
        stg = ctx.enter_context(tc.tile_pool(name="stg", bufs=2, space="PSUM"))
        psS = ctx.enter_context(tc.tile_pool(name="psS", bufs=3, space="PSUM"))
        psB = ctx.enter_context(tc.tile_pool(name="psB", bufs=1, space="PSUM"))
        psO = ctx.enter_context(tc.tile_pool(name="psO", bufs=1, space="PSUM"))

        # ---- stage constant/weight inputs into SBUF ----
        ident = persist.tile([128, 128], bf16, tag="ident")
        nc.sync.dma_start(ident[:], ins["ident_bf"])
        identf = persist.tile([65, 65], f32, tag="identf")
        nc.sync.dma_start(identf[:], ins["ident_f32"][0:65, 0:65])

        relT = persist.tile([128, 2056], bf16, tag="relT")
        nc.sync.dma_start(relT[:, 0:2049], ins["relTa"])

        # Wq/Wv [512, 256] -> [128, 4*256]
        wq_sb = persist.tile([128, 1024], bf16, tag="wq")
        wv_sb = persist.tile([128, 1024], bf16, tag="wv")
        for k in range(4):
            nc.sync.dma_start(wq_sb[:, k * 256:(k + 1) * 256],
                              ins["Wq"][k * 128:(k + 1) * 128, :])
            nc.sync.dma_start(wv_sb[:, k * 256:(k + 1) * 256],
                              ins["Wv"][k * 128:(k + 1) * 128, :])
        # qT/vT [512, 1024] -> 4 tiles each
        qT_sb, vT_sb = [], []
        for k in range(4):
            t = io.tile([128, 1024], bf16, tag=f"qT{k}", name=f"qTs{k}")
            nc.sync.dma_start(t[:], ins["qT"][k * 128:(k + 1) * 128, :])
            qT_sb.append(t)
        for k in range(4):
            t = io.tile([128, 1024], bf16, tag=f"vT{k}", name=f"vTs{k}")
            nc.sync.dma_start(t[:], ins["vT"][k * 128:(k + 1) * 128, :])
            vT_sb.append(t)
        # kT [256, 1024] -> 2 tiles
        kT_sb = []
        for t_ in range(2):
            t = persist.tile([128, 1024], bf16, tag=f"kT{t_}", name=f"kTs{t_}")
            nc.sync.dma_start(t[:], ins["kT"][t_ * 128:(t_ + 1) * 128, :])
            kT_sb.append(t)
        # small vectors
        bq_sb = persist.tile([128, 2], f32, tag="bq")
        nc.sync.dma_start(bq_sb[:], ins["bq2"])
        brr_sb = persist.tile([128, 2], f32, tag="brr")
        nc.sync.dma_start(brr_sb[:], ins["brr2"])
        brw_sb = persist.tile([128, 2], f32, tag="brw")
        nc.sync.dma_start(brw_sb[:], ins["brw2"])
        mb_sb = persist.tile([128, 8], f32, tag="mb")
        nc.sync.dma_start(mb_sb[:], ins["maskbias"])
        bv_sb = persist.tile([128, 256], f32, tag="bv")
        nc.sync.dma_start(bv_sb[:], ins["bv128"])

        biasA = persist.tile([128, 2], f32, tag="biasA")  # bq + r_r_bias
        nc.vector.tensor_add(biasA[:], bq_sb[:], brr_sb[:])
        biasB = persist.tile([128, 2], f32, tag="biasB")  # bq + r_w_bias
        nc.vector.tensor_add(biasB[:], bq_sb[:], brw_sb[:])

        # ---- q projection: qrrT/grwT [2 x (128, 1024)] (d on partitions) ----
        qrrT = [persist.tile([128, 1024], bf16, tag=f"qrrT{i}", name=f"qrrT{i}")
                for i in range(2)]
        grwT = [persist.tile([128, 1024], bf16, tag=f"grwT{i}", name=f"grwT{i}")
                for i in range(2)]

        def q_proj():
            for t_ in range(2):
                for nh in range(2):
                    ps = stg.tile([128, 512], f32, tag="ps")
                    for k in range(4):
                        nc.tensor.matmul(
                            ps[:],
                            wq_sb[:, k * 256 + t_ * 128: k * 256 + (t_ + 1) * 128],
                            qT_sb[k][:, nh * 512:(nh + 1) * 512],
                            start=(k == 0), stop=(k == 3),
                        )
                    nc.scalar.activation(qrrT[t_][:, nh * 512:(nh + 1) * 512], ps[:],
                                         Ident, bias=biasA[:, t_:t_ + 1], scale=1.0)
                    nc.scalar.activation(grwT[t_][:, nh * 512:(nh + 1) * 512], ps[:],
                                         Ident, bias=biasB[:, t_:t_ + 1], scale=1.0)

        # ---- v projection -> v_sb tiles [128, 4*65] ([v_head | 1]) ----
        v_sb = [persist.tile([128, 260], bf16, tag=f"vsb{lt}", name=f"vsb{lt}")
                for lt in range(8)]

        def v_proj(lt):
            ps = stg.tile([128, 512], f32, tag="ps")
            for k in range(4):
                nc.tensor.matmul(
                    ps[:, 0:256],
                    vT_sb[k][:, lt * 128:(lt + 1) * 128],
                    wv_sb[:, k * 256:(k + 1) * 256],
                    start=(k == 0), stop=(k == 3),
                )
            nc.vector.tensor_add(ps[:, 0:256], ps[:, 0:256], bv_sb[:])
            vt = v_sb[lt]
            src = ps[:, 0:256].rearrange("p (h d) -> p h d", d=64)
            dst = vt[:].rearrange("p (h e) -> p h e", e=65)[:, :, 0:64]
            nc.vector.tensor_copy(dst, src)
            nc.vector.memset(vt[:].rearrange("p (h e) -> p h e", e=65)[:, :, 64:65], 1.0)

        # ---- per-(b,head) pair machinery ----
        pair_state = {}
        copy_ctr = [0]

        def copy_out(dst, src):
            # alternate PSUM->SBUF copies between ACT and DVE
            if copy_ctr[0] % 2 == 0:
                nc.scalar.copy(dst, src)
            else:
                nc.vector.tensor_copy(dst, src)
            copy_ctr[0] += 1

        def produce_init(p):
            scrB = [scrp.tile([128, PITCH], bf16, tag=f"scrB{a}", name=f"scrB{a}_{p}")
                    for a in range(8)]
            scrE = [scrp.tile([128, PITCH], bf16, tag=f"scrE{a}", name=f"scrE{a}_{p}")
                    for a in range(8)]
            bdall = skp.tile([128, 8192], bf16, tag="bd", name=f"bdall{p}")
            eshall = skp.tile([128, 8192], bf16, tag="esh", name=f"esh{p}")
            c2 = sml.tile([128, 8], f32, tag="c2", name=f"c2_{p}")
            ebias = sml.tile([128, 8], f32, tag="ebias", name=f"ebias{p}")
            pair_state[p] = (scrB, scrE, bdall, eshall, c2, ebias)

        def produce_tile(p, src_sel, a):
            """One B_ (src_sel=0) or E_ (1) block of pair p -> SBUF scratch,
            followed immediately by its skewed re-read DMA."""
            t_ = p // 2
            o = (p % 2) * 64
            scrB, scrE, bdall, eshall, c2, ebias = pair_state[p]
            scr = (scrB if src_sel == 0 else scrE)[a]
            dstall = bdall if src_sel == 0 else eshall
            W0 = 897 - 128 * a
            if src_sel == 0:
                lhs = grwT[t_][o:o + 64, a * 128:(a + 1) * 128]
            else:
                lhs = kT_sb[t_][o:o + 64, a * 128:(a + 1) * 128]
            for ci, (c0, cw) in enumerate(((0, 512), (512, 512), (1024, 128))):
                ps = stg.tile([128, 512], f32, tag="ps")
                nc.tensor.matmul(ps[:, 0:cw], lhs,
                                 relT[o:o + 64, W0 + c0:W0 + c0 + cw],
                                 start=True, stop=True)
                if src_sel == 1 and ci == 2:
                    nc.tensor.matmul(ps[:, 128:129], lhs,
                                     relT[o:o + 64, 2048:2049],
                                     start=True, stop=True)
                    nc.scalar.activation(c2[:, a:a + 1], ps[:, 128:129],
                                         Ident, bias=0.0, scale=1.0)
                copy_out(scr[:, c0:c0 + cw], ps[:, 0:cw])
            # skewed re-read for this block: out[p_, j] = scr[p_, 127-p_+j]
            scr_ap = scr[:]
            diag = bass.AP(scr_ap.tensor, scr_ap.offset + 127,
                           [[PITCH - 1, 128], [1, 1024]])
            nc.sync.dma_start(dstall[:, a * 1024:(a + 1) * 1024], diag)

        def produce_fini(p):
            c2, ebias = pair_state[p][4], pair_state[p][5]
            nc.vector.tensor_add(ebias[:], c2[:], mb_sb[:])

        def ki_front(p, ki):
            """Transposes + S^T assembly + exp for both q-halves."""
            t_ = p // 2
            o = (p % 2) * 64
            scrB, scrE, bdall, eshall, c2, ebias = pair_state[p]
            kTs = kT_sb[t_]
            qrr = qrrT[t_]
            Ps = []
            BDT = psB.tile([128, 1024], bf16, tag="bdt")
            for h in range(2):
                for qb in range(4):
                    qi = 4 * h + qb
                    nc.tensor.matmul(
                        BDT[:, qi * 128:(qi + 1) * 128],
                        bdall[:, qi * 1024 + ki * 128: qi * 1024 + (ki + 1) * 128],
                        ident[:],
                        is_transpose=True, start=True, stop=True,
                    )
                s_h = psS.tile([128, 512], f32, tag="s")
                nc.vector.tensor_add(
                    s_h[:], BDT[:, h * 512:(h + 1) * 512],
                    eshall[:, ki * 1024 + h * 512: ki * 1024 + (h + 1) * 512])
                nc.tensor.matmul(
                    s_h[:],
                    kTs[o:o + 64, ki * 128:(ki + 1) * 128],
                    qrr[o:o + 64, h * 512:(h + 1) * 512],
                    start=False, stop=True, skip_group_check=True,
                )
                P = pp.tile([128, 512], bf16, tag="p")
                nc.scalar.activation(P[:], s_h[:], Exp,
                                     bias=ebias[:, ki:ki + 1], scale=1.0)
                Ps.append(P)
            return Ps

        def ki_back(p, ki, Ps):
            # outT[e, q]: e<64 = (sum_k P v)[d=e]; e=64 = sum_k P (denominator)
            outT = pair_state[("o", p)]
            for h in range(2):
                nc.tensor.matmul(
                    outT[0:65, h * 512:(h + 1) * 512],
                    v_sb[ki][:, p * 65:p * 65 + 65],
                    Ps[h][:],
                    start=(ki == 0), stop=(ki == 7),
                )

        def finalize(p):
            outT = pair_state[("o", p)]
            oT = denp.tile([65, 1024], f32, tag="oT", name=f"oT{p}")
            nc.scalar.copy(oT[:], outT[:])
            fin = finp.tile([128, 512], f32, tag="fin", name=f"fin{p}")
            for g in range(8):
                psF = stg.tile([128, 512], f32, tag="ps")
                nc.tensor.matmul(psF[0:128, 0:65],
                                 oT[0:65, g * 128:(g + 1) * 128],
                                 identf[0:65, 0:65],
                                 is_transpose=True, start=True, stop=True)
                nc.vector.tensor_scalar(fin[:, g * 64:(g + 1) * 64],
                                        psF[:, 0:64], psF[:, 64:65], None,
                                        op0=mybir.AluOpType.divide)
            dst = outs["out"][p].rearrange("(a r) d -> r a d", r=128)
            nc.sync.dma_start(dst, fin[:].rearrange("r (a d) -> r a d", d=64))

        def produce_block(p, idx, order):
            """Emit produce-block idx (0..15) of pair p per `order`: a list of
            (src_sel, a)."""
            if idx == 0:
                produce_init(p)
            src_sel, a = order[idx]
            produce_tile(p, src_sel, a)
            if idx == 15:
                produce_fini(p)

        # pair 0: one E block first (needs only kT/relT), then after q_proj all
        # B blocks (bdall gates ki 0); E[1..7] trail into the ki-loop since
        # eshall[a] is only needed at ki a. Pipelined pairs: B first.
        order0 = ([(1, 0)] + [(0, a) for a in range(8)] +
                  [(1, a) for a in range(1, 8)])
        orderP = [(0, a) for a in range(8)] + [(1, a) for a in range(8)]

        # ---- emission: head ----
        produce_block(0, 0, order0)
        q_proj()
        for idx in range(1, 9):
            produce_block(0, idx, order0)
        for lt in range(4):
            v_proj(lt)
        for idx in range(9, 13):
            produce_block(0, idx, order0)
        for lt in range(4, 8):
            v_proj(lt)
        for idx in range(13, 16):
            produce_block(0, idx, order0)

        # ---- steady state ----
        # ki_back (AV) runs one iteration behind ki_front so the in-order PE
        # stream never waits on the current ki's exp.
        for p in range(4):
            pair_state[("o", p)] = psO.tile([65, 1024], f32, tag="o", name=f"outT{p}")
            prev = None
            for ki in range(8):
                if p < 3:
                    produce_block(p + 1, 2 * ki, orderP)
                Ps = ki_front(p, ki)
                if p < 3:
                    produce_block(p + 1, 2 * ki + 1, orderP)
                if prev is not None:
                    ki_back(p, prev[0], prev[1])
                prev = (ki, Ps)
            ki_back(p, prev[0], prev[1])
            finalize(p)


_CACHE = {}


def _build_nc():
    if "nc" in _CACHE:
        return _CACHE["nc"]
    nc = bacc.Bacc("TRN2", target_bir_lowering=False, debug=False,
                   enable_asserts=False, num_devices=8)
    f32 = mybir.dt.float32
    bf16 = mybir.dt.bfloat16
    ins = {}

    def di(name, shape, dt):
        ins[name] = nc.dram_tensor(name, shape, dt, kind="ExternalInput").ap()

    di("qT", [512, 1024], bf16)
    di("vT", [512, 1024], bf16)
    di("kT", [256, 1024], bf16)
    di("Wq", [512, 256], bf16)
    di("Wv", [512, 256], bf16)
    di("relTa", [128, 2049], bf16)
    di("bq2", [128, 2], f32)
    di("brr2", [128, 2], f32)
    di("brw2", [128, 2], f32)
    di("maskbias", [128, 8], f32)
    di("bv128", [128, 256], f32)
    di("ident_bf", [128, 128], bf16)
    di("ident_f32", [128, 128], f32)
    outs = {"out": nc.dram_tensor("out", [4, 1024, 64], f32, kind="ExternalOutput").ap()}

    with tile.TileContext(nc) as tc:
        build_body(tc, ins, outs)
    nc.compile()
    _CACHE["nc"] = nc
    return nc


def make_in_maps(query, key, value, w_q_w, w_q_b, w_v_w, w_v_b, w_r_w, w_r_b,
                 r_r_bias, r_w_bias, seq_len):
    emb = _get_embedding(L, H)
    rel = (emb @ w_r_w.astype(np.float64) + w_r_b.astype(np.float64))  # [2L, 64]
    relTa = np.zeros((128, 2049), dtype=BF16)
    relTa[0:64, 0:2048] = rel.T.astype(BF16)
    relTa[0:64, 2048] = w_r_b.astype(BF16)
    relTa[64:128, :] = relTa[0:64, :]

    ident_bf = np.eye(128, dtype=BF16)
    seq_len = int(seq_len)
    in_maps = []
    for c in range(8):
        b, hg = c // 2, c % 2
        hs = 256 * hg
        heads = slice(4 * hg, 4 * hg + 4)
        mb = np.where((np.arange(1024) < seq_len), 0.0, NEG).astype(np.float32)
        in_maps.append({
            "qT": np.ascontiguousarray(query[b].T).astype(BF16),
            "vT": np.ascontiguousarray(value[b].T).astype(BF16),
            "kT": np.ascontiguousarray(key[b][:, hs:hs + 256].T).astype(BF16),
            "Wq": np.ascontiguousarray(w_q_w[:, hs:hs + 256]).astype(BF16),
            "Wv": np.ascontiguousarray(w_v_w[:, hs:hs + 256]).astype(BF16),
            "relTa": relTa,
            "bq2": np.ascontiguousarray(w_q_b[hs:hs + 256].reshape(2, 128).T).astype(np.float32),
            "brr2": np.ascontiguousarray(r_r_bias[heads].reshape(2, 128).T).astype(np.float32),
            "brw2": np.ascontiguousarray(r_w_bias[heads].reshape(2, 128).T).astype(np.float32),
            "maskbias": np.ascontiguousarray(mb.reshape(8, 128).T).astype(np.float32),
            "bv128": np.tile(w_v_b[hs:hs + 256][None, :], (128, 1)).astype(np.float32),
            "ident_bf": ident_bf,
            "ident_f32": np.eye(128, dtype=np.float32),
        })
    return in_maps


def kernel(query, key, value, w_q_w, w_q_b, w_v_w, w_v_b, w_r_w, w_r_b,
           r_r_bias, r_w_bias, seq_len, _trace=False):
    query = np.asarray(query); key = np.asarray(key); value = np.asarray(value)
    w_q_w = np.asarray(w_q_w); w_q_b = np.asarray(w_q_b)
    w_v_w = np.asarray(w_v_w); w_v_b = np.asarray(w_v_b)
    w_r_w = np.asarray(w_r_w); w_r_b = np.asarray(w_r_b)
    r_r_bias = np.asarray(r_r_bias); r_w_bias = np.asarray(r_w_bias)

    nc = _build_nc()
    in_maps = make_in_maps(query, key, value, w_q_w, w_q_b, w_v_w, w_v_b,
                           w_r_w, w_r_b, r_r_bias, r_w_bias, seq_len)
    res = run_bass_kernel_spmd(nc, in_maps, core_ids=list(range(8)), trace=_trace)
    out = np.zeros((B, L, H), dtype=np.float32)
    for c in range(8):
        b, hg = c // 2, c % 2
        o = res.results[c]["out"]  # [4, 1024, 64]
        for j in range(4):
            out[b][:, 256 * hg + 64 * j: 256 * hg + 64 * (j + 1)] = o[j]
    if _trace:
        return out, res
    return out


# revision 18
# speedup vs baseline: 1.0342x; 1.0342x over previous
"""Trainium2 Bass kernel for AdaptSelfAttention (Transformer-XL style relative
position attention).

Shapes (hardcoded): B=4, L=1024, H=512, NH=8, HD=64.
Sharding: 32 (batch, head) pairs -> 8 cores; core c handles batch c//2 and the
4-head group c%2 (hidden slice of 256 columns).

Math per (b, n):
  q = query @ Wq + bq   (per-head slice)          [L, 64]
  v = value @ Wv + bv                              [L, 64]
  k = key slice                                    [L, 64]
  rel = emb @ Wr + br                              [2L, 64]  (emb = sinusoid const)
  S[q_,k_] = (q+rrb).k  +  (q+rwb).rel[L+k_-q_]  +  k.rel[L+q_-k_]  + c2[k_]
       (c2[k_] = k.br ; the q-side br term is constant per row -> softmax-
        invariant, dropped)
  out = softmax_k(S with key-mask) @ v

v2 design (all shift scratch kept in SBUF; no DRAM round trip):
  - B_[q,l] / E_[k,l] windows ([128, 8*1152] bf16 per tensor per pair) are
    produced by PE into PSUM and copied straight into SBUF scratch.
  - Skewed (diagonal-AP) SBUF->SBUF DMAs, one per 128-row block, give
    bdall[q, k] = BD_sh and eshall = E_sh^T without touching HBM.
  - ki-loop: PE transposes BD blocks into a bf16 PSUM tile; one DVE add
    (BDT half + esh half) initializes the S^T PSUM; the AC matmul accumulates
    on top (start=False); exp on ACT with the key-mask/c2 bias; AV with
    lhsT=[v|1] so the softmax denominator is row 64 of the output PSUM.
  - finalize: DVE divide by the broadcast denominator row; output written
    [64, 1024] (transposed), host restores orientation.
Pairs are software-pipelined: pair p+1's produce+skew interleave pair p's
ki-loop at two produce-blocks per ki.
"""

import math
import sys

import numpy as np

sys.path.insert(0, "/opt/trn_rl_repo")

import concourse.bass as bass
import concourse.tile as tile
from concourse import bacc, mybir
from concourse.bass_utils import run_bass_kernel_spmd

import ml_dtypes

BF16 = ml_dtypes.bfloat16

B, L, H, NH, HD = 4, 1024, 512, 8, 64
PITCH = 1152  # stored l-window width per 128-row block of the B_/E_ scratch
SCRW = PITCH * 8
NEG = -1e30


def _get_embedding(max_len, dim):
    half = dim // 2
    freq = np.exp(np.arange(half, dtype=np.float64) * (-math.log(10000.0) / (half - 1)))
    pos = np.arange(-max_len, max_len, dtype=np.float64)
    ang = pos[:, None] * freq[None, :]
    return np.concatenate([np.sin(ang), np.cos(ang)], axis=1)


def build_body(tc, ins, outs):
    """Emit the per-core kernel. ins/outs: dicts of bass.AP over DRAM."""
    nc = tc.nc
    f32 = mybir.dt.float32
    bf16 = mybir.dt.bfloat16
    Ident = mybir.ActivationFunctionType.Identity
    Exp = mybir.ActivationFunctionType.Exp

    from contextlib import ExitStack

    ctx = ExitStack()
    with ctx:
        # ---- pools ----
        io = ctx.enter_context(tc.tile_pool(name="io", bufs=1))
        persist = ctx.enter_context(tc.tile_pool(name="persist", bufs=1))
        scrp = ctx.enter_context(tc.tile_pool(name="scrp", bufs=1))   # scratch windows
        skp = ctx.enter_context(tc.tile_pool(name="skp", bufs=2))     # skew landing
        pp = ctx.enter_context(tc.tile_pool(name="pp", bufs=4))       # exp outputs
        finp = ctx.enter_context(tc.tile_pool(name="finp", bufs=1))
        denp = ctx.enter_context(tc.tile_pool(name="denp", bufs=1))
        sml = ctx.enter_context(tc.tile_pool(name="sml", bufs=4))
        # PSUM: stg 2x[128,512]f32=2 banks; psS 3x[128,512]f32=3;
        # psB 2x[128,512]bf16=1; psO 1x[65,1024]f32=2  -> 8 banks
        stg = ctx.enter_context(tc.tile_pool(name="stg", bufs=2, space="PSUM"))
        psS = ctx.enter_context(tc.tile_pool(name="psS", bufs=2, space="PSUM"))
        psB = ctx.enter_context(tc.tile_pool(name="psB", bufs=2, space="PSUM"))
        psO = ctx.enter_context(tc.tile_pool(name="psO", bufs=1, space="PSUM"))

        # ---- stage constant/weight inputs into SBUF ----
        ident = persist.tile([128, 128], bf16, tag="ident")
        nc.sync.dma_start(ident[:], ins["ident_bf"])
        identf = persist.tile([65, 65], f32, tag="identf")
        nc.sync.dma_start(identf[:], ins["ident_f32"][0:65, 0:65])

        relT = persist.tile([128, 2056], bf16, tag="relT")
        nc.sync.dma_start(relT[:, 0:2049], ins["relTa"])

        # Wq/Wv [512, 256] -> [128, 4*256]
        wq_sb = persist.tile([128, 1024], bf16, tag="wq")
        wv_sb = persist.tile([128, 1024], bf16, tag="wv")
        for k in range(4):
            nc.sync.dma_start(wq_sb[:, k * 256:(k + 1) * 256],
                              ins["Wq"][k * 128:(k + 1) * 128, :])
            nc.sync.dma_start(wv_sb[:, k * 256:(k + 1) * 256],
                              ins["Wv"][k * 128:(k + 1) * 128, :])
        # qT/vT [512, 1024] -> 4 tiles each
        qT_sb, vT_sb = [], []
        for k in range(4):
            t = io.tile([128, 1024], bf16, tag=f"qT{k}", name=f"qTs{k}")
            nc.sync.dma_start(t[:], ins["qT"][k * 128:(k + 1) * 128, :])
            qT_sb.append(t)
        for k in range(4):
            t = io.tile([128, 1024], bf16, tag=f"vT{k}", name=f"vTs{k}")
            nc.sync.dma_start(t[:], ins["vT"][k * 128:(k + 1) * 128, :])
            vT_sb.append(t)
        # kT [256, 1024] -> 2 tiles
        kT_sb = []
        for t_ in range(2):
            t = persist.tile([128, 1024], bf16, tag=f"kT{t_}", name=f"kTs{t_}")
            nc.sync.dma_start(t[:], ins["kT"][t_ * 128:(t_ + 1) * 128, :])
            kT_sb.append(t)
        # small vectors
        bq_sb = persist.tile([128, 2], f32, tag="bq")
        nc.sync.dma_start(bq_sb[:], ins["bq2"])
        brr_sb = persist.tile([128, 2], f32, tag="brr")
        nc.sync.dma_start(brr_sb[:], ins["brr2"])
        brw_sb = persist.tile([128, 2], f32, tag="brw")
        nc.sync.dma_start(brw_sb[:], ins["brw2"])
        mb_sb = persist.tile([128, 8], f32, tag="mb")
        nc.sync.dma_start(mb_sb[:], ins["maskbias"])
        bv_sb = persist.tile([128, 256], f32, tag="bv")
        nc.sync.dma_start(bv_sb[:], ins["bv128"])

        biasA = persist.tile([128, 2], f32, tag="biasA")  # bq + r_r_bias
        nc.vector.tensor_add(biasA[:], bq_sb[:], brr_sb[:])
        biasB = persist.tile([128, 2], f32, tag="biasB")  # bq + r_w_bias
        nc.vector.tensor_add(biasB[:], bq_sb[:], brw_sb[:])

        # ---- q projection: qrrT/grwT [2 x (128, 1024)] (d on partitions) ----
        qrrT = [persist.tile([128, 1024], bf16, tag=f"qrrT{i}", name=f"qrrT{i}")
                for i in range(2)]
        grwT = [persist.tile([128, 1024], bf16, tag=f"grwT{i}", name=f"grwT{i}")
                for i in range(2)]

        def q_proj():
            for t_ in range(2):
                for nh in range(2):
                    ps = stg.tile([128, 512], f32, tag="ps")
                    for k in range(4):
                        nc.tensor.matmul(
                            ps[:],
                            wq_sb[:, k * 256 + t_ * 128: k * 256 + (t_ + 1) * 128],
                            qT_sb[k][:, nh * 512:(nh + 1) * 512],
                            start=(k == 0), stop=(k == 3),
                        )
                    nc.scalar.activation(qrrT[t_][:, nh * 512:(nh + 1) * 512], ps[:],
                                         Ident, bias=biasA[:, t_:t_ + 1], scale=1.0)
                    nc.scalar.activation(grwT[t_][:, nh * 512:(nh + 1) * 512], ps[:],
                                         Ident, bias=biasB[:, t_:t_ + 1], scale=1.0)

        # ---- v projection -> v_sb tiles [128, 4*65] ([v_head | 1]) ----
        v_sb = [persist.tile([128, 260], bf16, tag=f"vsb{lt}", name=f"vsb{lt}")
                for lt in range(8)]

        def v_proj(lt):
            ps = stg.tile([128, 512], f32, tag="ps")
            for k in range(4):
                nc.tensor.matmul(
                    ps[:, 0:256],
                    vT_sb[k][:, lt * 128:(lt + 1) * 128],
                    wv_sb[:, k * 256:(k + 1) * 256],
                    start=(k == 0), stop=(k == 3),
                )
            nc.vector.tensor_add(ps[:, 0:256], ps[:, 0:256], bv_sb[:])
            vt = v_sb[lt]
            src = ps[:, 0:256].rearrange("p (h d) -> p h d", d=64)
            dst = vt[:].rearrange("p (h e) -> p h e", e=65)[:, :, 0:64]
            nc.vector.tensor_copy(dst, src)
            nc.vector.memset(vt[:].rearrange("p (h e) -> p h e", e=65)[:, :, 64:65], 1.0)

        # ---- per-(b,head) pair machinery ----
        pair_state = {}
        copy_ctr = [0]

        def copy_out(dst, src):
            # alternate PSUM->SBUF copies between ACT and DVE
            if copy_ctr[0] % 2 == 0:
                nc.scalar.copy(dst, src)
            else:
                nc.vector.tensor_copy(dst, src)
            copy_ctr[0] += 1

        def produce_init(p):
            scrB = [scrp.tile([128, PITCH], bf16, tag=f"scrB{a}", name=f"scrB{a}_{p}")
                    for a in range(8)]
            scrE = [scrp.tile([128, PITCH], bf16, tag=f"scrE{a}", name=f"scrE{a}_{p}")
                    for a in range(8)]
            bdall = skp.tile([128, 8192], bf16, tag="bd", name=f"bdall{p}")
            eshall = skp.tile([128, 8192], bf16, tag="esh", name=f"esh{p}")
            c2 = sml.tile([128, 8], f32, tag="c2", name=f"c2_{p}")
            ebias = sml.tile([128, 8], f32, tag="ebias", name=f"ebias{p}")
            pair_state[p] = (scrB, scrE, bdall, eshall, c2, ebias)

        def produce_tile(p, src_sel, a):
            """One B_ (src_sel=0) or E_ (1) block of pair p -> SBUF scratch,
            followed immediately by its skewed re-read DMA."""
            t_ = p // 2
            o = (p % 2) * 64
            scrB, scrE, bdall, eshall, c2, ebias = pair_state[p]
            scr = (scrB if src_sel == 0 else scrE)[a]
            dstall = bdall if src_sel == 0 else eshall
            W0 = 897 - 128 * a
            if src_sel == 0:
                lhs = grwT[t_][o:o + 64, a * 128:(a + 1) * 128]
            else:
                lhs = kT_sb[t_][o:o + 64, a * 128:(a + 1) * 128]
            for ci, (c0, cw) in enumerate(((0, 512), (512, 512), (1024, 128))):
                ps = stg.tile([128, 512], f32, tag="ps")
                nc.tensor.matmul(ps[:, 0:cw], lhs,
                                 relT[o:o + 64, W0 + c0:W0 + c0 + cw],
                                 start=True, stop=True)
                if src_sel == 1 and ci == 2:
                    nc.tensor.matmul(ps[:, 128:129], lhs,
                                     relT[o:o + 64, 2048:2049],
                                     start=True, stop=True)
                    nc.scalar.activation(c2[:, a:a + 1], ps[:, 128:129],
                                         Ident, bias=0.0, scale=1.0)
                copy_out(scr[:, c0:c0 + cw], ps[:, 0:cw])
            # skewed re-read for this block: out[p_, j] = scr[p_, 127-p_+j]
            scr_ap = scr[:]
            diag = bass.AP(scr_ap.tensor, scr_ap.offset + 127,
                           [[PITCH - 1, 128], [1, 1024]])
            nc.sync.dma_start(dstall[:, a * 1024:(a + 1) * 1024], diag)

        def produce_fini(p):
            c2, ebias = pair_state[p][4], pair_state[p][5]
            nc.vector.tensor_add(ebias[:], c2[:], mb_sb[:])

        def ki_front(p, ki):
            """Transposes + S^T assembly + exp for both q-halves."""
            t_ = p // 2
            o = (p % 2) * 64
            scrB, scrE, bdall, eshall, c2, ebias = pair_state[p]
            kTs = kT_sb[t_]
            qrr = qrrT[t_]
            Ps = []
            for h in range(2):
                BDT = psB.tile([128, 512], bf16, tag="bdt")
                for qb in range(4):
                    qi = 4 * h + qb
                    nc.tensor.matmul(
                        BDT[:, qb * 128:(qb + 1) * 128],
                        bdall[:, qi * 1024 + ki * 128: qi * 1024 + (ki + 1) * 128],
                        ident[:],
                        is_transpose=True, start=True, stop=True,
                    )
                s_h = psS.tile([128, 512], f32, tag="s")
                nc.vector.tensor_add(
                    s_h[:], BDT[:],
                    eshall[:, ki * 1024 + h * 512: ki * 1024 + (h + 1) * 512])
                nc.tensor.matmul(
                    s_h[:],
                    kTs[o:o + 64, ki * 128:(ki + 1) * 128],
                    qrr[o:o + 64, h * 512:(h + 1) * 512],
                    start=False, stop=True, skip_group_check=True,
                )
                P = pp.tile([128, 512], bf16, tag="p")
                nc.scalar.activation(P[:], s_h[:], Exp,
                                     bias=ebias[:, ki:ki + 1], scale=1.0)
                Ps.append(P)
            return Ps

        def ki_back(p, ki, Ps):
            # outT[e, q]: e<64 = (sum_k P v)[d=e]; e=64 = sum_k P (denominator)
            outT = pair_state[("o", p)]
            for h in range(2):
                nc.tensor.matmul(
                    outT[0:65, h * 512:(h + 1) * 512],
                    v_sb[ki][:, p * 65:p * 65 + 65],
                    Ps[h][:],
                    start=(ki == 0), stop=(ki == 7),
                )

        def finalize(p):
            outT = pair_state[("o", p)]
            oT = denp.tile([65, 1024], f32, tag="oT", name=f"oT{p}")
            nc.scalar.copy(oT[:], outT[:])
            fin = finp.tile([128, 512], f32, tag="fin", name=f"fin{p}")
            for g in range(8):
                psF = stg.tile([128, 512], f32, tag="ps")
                nc.tensor.matmul(psF[0:128, 0:65],
                                 oT[0:65, g * 128:(g + 1) * 128],
                                 identf[0:65, 0:65],
                                 is_transpose=True, start=True, stop=True)
                nc.vector.tensor_scalar(fin[:, g * 64:(g + 1) * 64],
                                        psF[:, 0:64], psF[:, 64:65], None,
                                        op0=mybir.AluOpType.divide)
            dst = outs["out"][p].rearrange("(a r) d -> r a d", r=128)
            nc.sync.dma_start(dst, fin[:].rearrange("r (a d) -> r a d", d=64))

        def produce_block(p, idx, order):
            """Emit produce-block idx (0..15) of pair p per `order`: a list of
            (src_sel, a)."""
            if idx == 0:
                produce_init(p)
            src_sel, a = order[idx]
            produce_tile(p, src_sel, a)
            if idx == 15:
                produce_fini(p)

        # pair 0: E first (only needs kT/relT input DMAs); pipelined pairs:
        # B first (bdall needed at ki 0, eshall[a] only at ki a).
        order0 = [(1, a) for a in range(8)] + [(0, a) for a in range(8)]
        orderP = [(0, a) for a in range(8)] + [(1, a) for a in range(8)]

        # ---- emission: head ----
        for idx in range(8):
            produce_block(0, idx, order0)
        q_proj()
        for idx in range(8, 16):
            produce_block(0, idx, order0)
        for lt in range(8):
            v_proj(lt)

        # ---- steady state ----
        # ki_back (AV) runs one iteration behind ki_front so the in-order PE
        # stream never waits on the current ki's exp.
        for p in range(4):
            pair_state[("o", p)] = psO.tile([65, 1024], f32, tag="o", name=f"outT{p}")
            prev = None
            for ki in range(8):
                if p < 3:
                    produce_block(p + 1, 2 * ki, orderP)
                Ps = ki_front(p, ki)
                if p < 3:
                    produce_block(p + 1, 2 * ki + 1, orderP)
                if prev is not None:
                    ki_back(p, prev[0], prev[1])
                prev = (ki, Ps)
            ki_back(p, prev[0], prev[1])
            finalize(p)


_CACHE = {}


def _build_nc():
    if "nc" in _CACHE:
        return _CACHE["nc"]
    nc = bacc.Bacc("TRN2", target_bir_lowering=False, debug=False,
                   enable_asserts=False, num_devices=8)
    f32 = mybir.dt.float32
    bf16 = mybir.dt.bfloat16
    ins = {}

    def di(name, shape, dt):
        ins[name] = nc.dram_tensor(name, shape, dt, kind="ExternalInput").ap()

    di("qT", [512, 1024], bf16)
    di("vT", [512, 1024], bf16)
    di("kT", [256, 1024], bf16)
    di("Wq", [512, 256], bf16)
    di("Wv", [512, 256], bf16)
    di("relTa", [128, 2049], bf16)
    di("bq2", [128, 2], f32)
    di("brr2", [128, 2], f32)
    di("brw2", [128, 2], f32)
    di("maskbias", [128, 8], f32)
    di("bv128", [128, 256], f32)
    di("ident_bf", [128, 128], bf16)
    di("ident_f32", [128, 128], f32)
    outs = {"out": nc.dram_tensor("out", [4, 1024, 64], f32, kind="ExternalOutput").ap()}

    with tile.TileContext(nc) as tc:
        build_body(tc, ins, outs)
    nc.compile()
    _CACHE["nc"] = nc
    return nc


def make_in_maps(query, key, value, w_q_w, w_q_b, w_v_w, w_v_b, w_r_w, w_r_b,
                 r_r_bias, r_w_bias, seq_len):
    emb = _get_embedding(L, H)
    rel = (emb @ w_r_w.astype(np.float64) + w_r_b.astype(np.float64))  # [2L, 64]
    relTa = np.zeros((128, 2049), dtype=BF16)
    relTa[0:64, 0:2048] = rel.T.astype(BF16)
    relTa[0:64, 2048] = w_r_b.astype(BF16)
    relTa[64:128, :] = relTa[0:64, :]

    ident_bf = np.eye(128, dtype=BF16)
    seq_len = int(seq_len)
    in_maps = []
    for c in range(8):
        b, hg = c // 2, c % 2
        hs = 256 * hg
        heads = slice(4 * hg, 4 * hg + 4)
        mb = np.where((np.arange(1024) < seq_len), 0.0, NEG).astype(np.float32)
        in_maps.append({
            "qT": np.ascontiguousarray(query[b].T).astype(BF16),
            "vT": np.ascontiguousarray(value[b].T).astype(BF16),
            "kT": np.ascontiguousarray(key[b][:, hs:hs + 256].T).astype(BF16),
            "Wq": np.ascontiguousarray(w_q_w[:, hs:hs + 256]).astype(BF16),
            "Wv": np.ascontiguousarray(w_v_w[:, hs:hs + 256]).astype(BF16),
            "relTa": relTa,
            "bq2": np.ascontiguousarray(w_q_b[hs:hs + 256].reshape(2, 128).T).astype(np.float32),
            "brr2": np.ascontiguousarray(r_r_bias[heads].reshape(2, 128).T).astype(np.float32),
            "brw2": np.ascontiguousarray(r_w_bias[heads].reshape(2, 128).T).astype(np.float32),
            "maskbias": np.ascontiguousarray(mb.reshape(8, 128).T).astype(np.float32),
            "bv128": np.tile(w_v_b[hs:hs + 256][None, :], (128, 1)).astype(np.float32),
            "ident_bf": ident_bf,
            "ident_f32": np.eye(128, dtype=np.float32),
        })
    return in_maps


def kernel(query, key, value, w_q_w, w_q_b, w_v_w, w_v_b, w_r_w, w_r_b,
           r_r_bias, r_w_bias, seq_len, _trace=False):
    query = np.asarray(query); key = np.asarray(key); value = np.asarray(value)
    w_q_w = np.asarray(w_q_w); w_q_b = np.asarray(w_q_b)
    w_v_w = np.asarray(w_v_w); w_v_b = np.asarray(w_v_b)
    w_r_w = np.asarray(w_r_w); w_r_b = np.asarray(w_r_b)
    r_r_bias = np.asarray(r_r_bias); r_w_bias = np.asarray(r_w_bias)

    nc = _build_nc()
    in_maps = make_in_maps(query, key, value, w_q_w, w_q_b, w_v_w, w_v_b,
                           w_r_w, w_r_b, r_r_bias, r_w_bias, seq_len)
    res = run_bass_kernel_spmd(nc, in_maps, core_ids=list(range(8)), trace=_trace)
    out = np.zeros((B, L, H), dtype=np.float32)
    for c in range(8):
        b, hg = c // 2, c % 2
        o = res.results[c]["out"]  # [4, 1024, 64]
        for j in range(4):
            out[b][:, 256 * hg + 64 * j: 256 * hg + 64 * (j + 1)] = o[j]
    if _trace:
        return out, res
    return out
